# revision 1
# baseline (speedup 1.0000x reference)
"""GroupViT cross-attention layer on 8 TRN2 NeuronCores.

Strategy: pure data-parallel over batch (16 batches -> 2 per core, zero
collectives). Feature-major ("transposed") layout on chip: activations
stored [feature(partition), token(free)], weights host-transposed to
[d_in, d_out] so every matmul contracts over the partition dim.

dtypes: attention path bf16 (its output is ~1% of the residual stream,
errors diluted ~86x), MLP/residual/LN path float32r (~1e-4 matmul error
at full PE speed for free-dim >= 256).

Softmax: scores are O(+-3) so exp needs no max subtraction. Scores are
computed transposed [s, t]; denominators come free from a ones column
appended to V in the ctx matmul; normalization via a k=1 outer-product
broadcast matmul + one DVE multiply per head.

LN over the partition (feature) dim: sums via ones-column matmuls,
(x - mu)*rs*g + b applied as x (*) (g (x) rs) - (g (x) mu*rs - b (x) 1)
with the broadcast tensors built by tiny k=1 matmuls into PSUM.
"""

import numpy as np

B, T, S, D, H, HD, FF = 16, 512, 2048, 768, 12, 64, 3072
NCORES = 8
BPC = B // NCORES      # batches per core
P = 128
DC = D // P            # 6 feature chunks
SC = S // P            # 16 key-token chunks
FFC = FF // P          # 24
EPS = 1e-5
SCALE = HD ** -0.5

_cached = {}


def _build(use_bv: bool):
    import concourse.bacc as bacc
    import concourse.tile as tile
    import concourse.mybir as mybir

    f32 = mybir.dt.float32
    f32r = mybir.dt.float32r
    bf16 = mybir.dt.bfloat16
    AF = mybir.ActivationFunctionType
    ALU = mybir.AluOpType

    nc = bacc.Bacc("TRN2", target_bir_lowering=False, debug=False,
                   num_devices=NCORES)

    # ---- DRAM I/O (per-core shapes) ----
    qT_d = nc.dram_tensor("qT", [BPC, D, T], f32r, kind="ExternalInput")
    kT_d = nc.dram_tensor("kT", [BPC, D, S], f32, kind="ExternalInput")
    wq_d = nc.dram_tensor("wq_t", [D, D], f32r, kind="ExternalInput")
    wk_d = nc.dram_tensor("wk_t", [D, D], f32, kind="ExternalInput")
    wv_d = nc.dram_tensor("wv_t", [D, D], f32, kind="ExternalInput")
    wo_d = nc.dram_tensor("wo_t", [D, D], f32, kind="ExternalInput")
    fc1_d = nc.dram_tensor("fc1_t", [D, FF], f32r, kind="ExternalInput")
    fc2_d = nc.dram_tensor("fc2_t", [FF, D], f32r, kind="ExternalInput")
    bq_d = nc.dram_tensor("bqv", [D], f32, kind="ExternalInput")
    bk_d = nc.dram_tensor("bkv", [D], f32, kind="ExternalInput")
    bv_d = nc.dram_tensor("bvv", [1, D], f32r, kind="ExternalInput")
    bo_d = nc.dram_tensor("bov", [D], f32, kind="ExternalInput")
    f1b_d = nc.dram_tensor("f1b", [FF], f32, kind="ExternalInput")
    f2b_d = nc.dram_tensor("f2b", [D], f32, kind="ExternalInput")
    ln2g_d = nc.dram_tensor("ln2g", [1, D], f32r, kind="ExternalInput")
    ln2bn_d = nc.dram_tensor("ln2bn", [1, D], f32r, kind="ExternalInput")
    lnpg_d = nc.dram_tensor("lnpg", [1, D], f32r, kind="ExternalInput")
    lnpbn_d = nc.dram_tensor("lnpbn", [1, D], f32r, kind="ExternalInput")
    ones_col_d = nc.dram_tensor("ones_col", [P, 1], f32r, kind="ExternalInput")
    ones_row_d = nc.dram_tensor("ones_row", [1, T], f32r, kind="ExternalInput")
    out_d = nc.dram_tensor("out", [BPC, D, T], f32, kind="ExternalOutput")

    def F(ap):
        return ap.bitcast(f32)

    def act_reciprocal(out, in_):
        eng = nc.scalar
        ins = [eng.lower_ap(in_)]
        for v in (0.0, 1.0, 0.0):
            ins.append(mybir.ImmediateValue(dtype=f32, value=v))
        return eng.add_instruction(mybir.InstActivation(
            name=nc.get_next_instruction_name(),
            func=AF.Reciprocal, ins=ins, outs=[eng.lower_ap(out)]))

    with tile.TileContext(nc) as tc:
        with (
            tc.tile_pool(name="act", bufs=3) as act,
            tc.tile_pool(name="bigk", bufs=1) as bigk,
            tc.tile_pool(name="vpool", bufs=1) as vpool,
            tc.tile_pool(name="qtp", bufs=1) as qtp,
            tc.tile_pool(name="ktc", bufs=2) as ktc,
            tc.tile_pool(name="wstream", bufs=2) as wstream,
            tc.tile_pool(name="wvp", bufs=1) as wvp,
            tc.tile_pool(name="fstream", bufs=2) as fstream,
            tc.tile_pool(name="expp", bufs=3) as expp,
            tc.tile_pool(name="mchunk", bufs=3) as mchunkp,
            tc.tile_pool(name="tmp", bufs=3) as tmpp,
            tc.tile_pool(name="small", bufs=1) as small,
        ):
            # ---- persistent small tiles ----
            ones_col = small.tile([P, 1], f32r, tag="ones_col")
            nc.sync.dma_start(ones_col[:], ones_col_d.ap())
            ones_row = small.tile([1, T], f32r, tag="ones_row")
            nc.sync.dma_start(ones_row[:], ones_row_d.ap())
            ones64_f = small.tile([1, HD], f32, tag="ones64f")
            nc.vector.memset(ones64_f[:], 1.0)
            ones64_bf = small.tile([1, HD], bf16, tag="ones64")
            nc.vector.tensor_copy(ones64_bf[:], ones64_f[:])
            onesc_f = small.tile([P, 1], f32, tag="onesc_f")
            nc.vector.memset(onesc_f[:], 1.0)
            eps_t = small.tile([1, 1], f32, tag="eps")
            nc.vector.memset(eps_t[:], EPS)

            ln2g = small.tile([1, D], f32r, tag="ln2g")
            nc.sync.dma_start(ln2g[:], ln2g_d.ap())
            ln2bn = small.tile([1, D], f32r, tag="ln2bn")
            nc.sync.dma_start(ln2bn[:], ln2bn_d.ap())
            lnpg = small.tile([1, D], f32r, tag="lnpg")
            nc.sync.dma_start(lnpg[:], lnpg_d.ap())
            lnpbn = small.tile([1, D], f32r, tag="lnpbn")
            nc.sync.dma_start(lnpbn[:], lnpbn_d.ap())

            bq_pc = small.tile([P, DC], f32, tag="bq_pc")
            nc.sync.dma_start(bq_pc[:], bq_d.ap().rearrange("(c p) -> p c", p=P))
            bk_pc = small.tile([P, DC], f32, tag="bk_pc")
            nc.sync.dma_start(bk_pc[:], bk_d.ap().rearrange("(c p) -> p c", p=P))
            bo_pc = small.tile([P, DC], f32, tag="bo_pc")
            nc.sync.dma_start(bo_pc[:], bo_d.ap().rearrange("(c p) -> p c", p=P))
            f1b_pc = small.tile([P, FFC], f32, tag="f1b_pc")
            nc.sync.dma_start(f1b_pc[:], f1b_d.ap().rearrange("(c p) -> p c", p=P))
            f2b_pc = small.tile([P, DC], f32, tag="f2b_pc")
            nc.sync.dma_start(f2b_pc[:], f2b_d.ap().rearrange("(c p) -> p c", p=P))

            bv_row = None
            if use_bv:
                bv_row = small.tile([1, D], f32r, tag="bv_row")
                nc.sync.dma_start(bv_row[:], bv_d.ap())

            def ln_pass(xsrc, dst, g_row, bn_row, ps_scope):
                """LayerNorm over the partition(feature) dim:
                xsrc [P, DC, T] f32r -> dst [P, DC, T]."""
                ps_st, ps_bc = ps_scope
                psum_mu = ps_st.tile([1, T], f32, tag="st_mu")
                psum_sq = ps_st.tile([1, T], f32, tag="st_sq")
                for c in range(DC):
                    nc.tensor.matmul(psum_mu[:], ones_col[:], xsrc[:, c, :],
                                     start=(c == 0), stop=(c == DC - 1))
                sqt = []
                for c in range(DC):
                    sq = tmpp.tile([P, T], f32r, tag="lnsq")
                    nc.vector.tensor_mul(sq[:], F(xsrc[:, c, :]),
                                         F(xsrc[:, c, :]))
                    sqt.append(sq)
                for c in range(DC):
                    nc.tensor.matmul(psum_sq[:], ones_col[:], sqt[c][:],
                                     start=(c == 0), stop=(c == DC - 1))
                mu_f = small.tile([1, T], f32, tag="ln_mu")
                nc.vector.tensor_scalar_mul(mu_f[:], psum_mu[:], 1.0 / D)
                mu2_f = small.tile([1, T], f32, tag="ln_mu2")
                nc.vector.tensor_tensor(mu2_f[:], mu_f[:], mu_f[:], ALU.mult)
                var_f = small.tile([1, T], f32, tag="ln_var")
                nc.vector.scalar_tensor_tensor(
                    var_f[:], psum_sq[:], 1.0 / D, mu2_f[:],
                    op0=ALU.mult, op1=ALU.subtract)
                rs_f = small.tile([1, T], f32, tag="ln_rs")
                nc.scalar.activation(rs_f[:], var_f[:], AF.Abs_reciprocal_sqrt,
                                     bias=eps_t[:])
                rs_r = small.tile([1, T], f32r, tag="ln_rs_r")
                nc.vector.tensor_copy(rs_r[:], rs_f[:])
                mrs_r = small.tile([1, T], f32r, tag="ln_mrs_r")
                nc.vector.tensor_tensor(mrs_r[:], mu_f[:], rs_f[:], ALU.mult)
                for c in range(DC):
                    bcA = ps_bc.tile([P, T], f32, tag="ln_bcA")
                    bcB = ps_bc.tile([P, T], f32, tag="ln_bcB")
                    gsl = g_row[:, c * P:(c + 1) * P]
                    bsl = bn_row[:, c * P:(c + 1) * P]
                    nc.tensor.matmul(bcA[:], gsl, rs_r[:], start=True, stop=True)
                    nc.tensor.matmul(bcB[:], gsl, mrs_r[:], start=True, stop=False)
                    nc.tensor.matmul(bcB[:], bsl, ones_row[:], start=False, stop=True)
                    tmp = tmpp.tile([P, T], f32, tag="ln_tmp")
                    nc.vector.tensor_tensor(tmp[:], F(xsrc[:, c, :]), bcA[:],
                                            ALU.mult)
                    nc.vector.tensor_tensor(dst[:, c, :], tmp[:], bcB[:],
                                            ALU.subtract)

            for b in range(BPC):
                # ================= phase A: load + Q/V projections ======
                qin = act.tile([P, DC, T], f32r, tag="act")
                nc.sync.dma_start(qin[:], qT_d.ap()[b].rearrange(
                    "(c p) t -> p c t", p=P))
                kin = bigk.tile([P, DC, S], bf16, tag="kin")
                nc.gpsimd.dma_start(kin[:], kT_d.ap()[b].rearrange(
                    "(c p) s -> p c s", p=P))
                wv_sb = wvp.tile([P, DC, D], bf16, tag="wv")
                nc.gpsimd.dma_start(wv_sb[:], wv_d.ap().rearrange(
                    "(k p) o -> p k o", p=P))

                qt = qtp.tile([P, DC, T], bf16, tag="qt")
                with tc.tile_pool(name="psA", bufs=2, space="PSUM") as psA:
                    for mo in range(DC):
                        wq_sl = wstream.tile([P, DC, P], f32r, tag="wq_sl")
                        nc.sync.dma_start(wq_sl[:], wq_d.ap().rearrange(
                            "(k p) o -> p k o", p=P)[:, :, mo * P:(mo + 1) * P])
                        ps = psA.tile([P, T], f32, tag="psA")
                        for ki in range(DC):
                            nc.tensor.matmul(ps[:], wq_sl[:, ki, :],
                                             qin[:, ki, :],
                                             start=(ki == 0), stop=(ki == DC - 1))
                        nc.vector.tensor_scalar_add(qt[:, mo, :], ps[:],
                                                    bq_pc[:, mo:mo + 1])

                    v_sb = vpool.tile([P, SC, H, HD + 1], bf16, tag="v")
                    nc.vector.tensor_copy(
                        v_sb[:, :, :, HD:HD + 1],
                        onesc_f[:].to_broadcast([P, SC, H, 1]))
                    bv_bc = None
                    if use_bv:
                        bv_bc = small.tile([P, D], f32, tag="bv_bc")
                        for half in range(2):
                            ps_bv = psA.tile([P, 384], f32, tag="psA")
                            nc.tensor.matmul(
                                ps_bv[:], ones_row[:, 0:P],
                                bv_row[:, half * 384:(half + 1) * 384],
                                start=True, stop=True)
                            nc.vector.tensor_copy(
                                bv_bc[:, half * 384:(half + 1) * 384], ps_bv[:])
                    for so in range(SC):
                        for half in range(2):
                            ps = psA.tile([P, 384], f32, tag="psA")
                            for ki in range(DC):
                                nc.tensor.matmul(
                                    ps[:],
                                    kin[:, ki, so * P:(so + 1) * P],
                                    wv_sb[:, ki, half * 384:(half + 1) * 384],
                                    start=(ki == 0), stop=(ki == DC - 1))
                            dstv = v_sb[:, so, half * 6:(half + 1) * 6, 0:HD]
                            if use_bv:
                                nc.vector.tensor_tensor(
                                    dstv, ps[:],
                                    bv_bc[:, half * 384:(half + 1) * 384],
                                    ALU.add)
                            else:
                                nc.vector.tensor_copy(dstv, ps[:])

                # ================= phase B: attention ====================
                ctxT = act.tile([P, DC, T], bf16, tag="act")

                def attn_kproj(hp, kin, psK):
                    wk_sl = wstream.tile([P, DC, P], bf16, tag="wk_sl")
                    nc.gpsimd.dma_start(wk_sl[:], wk_d.ap().rearrange(
                        "(k p) o -> p k o", p=P)[:, :, hp * P:(hp + 1) * P])
                    ktch = ktc.tile([P, S], bf16, tag="ktc")
                    for no in range(4):
                        ps = psK.tile([P, T], f32, tag="psK")
                        for ki in range(DC):
                            nc.tensor.matmul(
                                ps[:], wk_sl[:, ki, :],
                                kin[:, ki, no * T:(no + 1) * T],
                                start=(ki == 0), stop=(ki == DC - 1))
                        nc.vector.tensor_scalar_add(
                            ktch[:, no * T:(no + 1) * T], ps[:],
                            bk_pc[:, hp:hp + 1])
                    return ktch

                def attn_scores_ctx(hp, so2, ktch, qt, v_sb, ps_ctx, psSC):
                    scs = []
                    for hh in range(2):
                        base = hh * HD
                        ps_sc = psSC.tile([P, 2 * T], f32, tag="psSC",
                                          name=f"ps_sc{hh}")
                        for j in range(2):
                            so = so2 + j
                            nc.tensor.matmul(
                                ps_sc[:, j * T:(j + 1) * T],
                                ktch[base:base + HD, so * P:(so + 1) * P],
                                qt[base:base + HD, hp, :],
                                start=True, stop=True)
                        scs.append(ps_sc)
                    exs = []
                    for hh in range(2):
                        ex = expp.tile([P, 2 * T], bf16, tag="exp",
                                       name=f"ex{hh}")
                        nc.scalar.activation(ex[:], scs[hh][:], AF.Exp)
                        exs.append(ex)
                    for hh in range(2):
                        h = 2 * hp + hh
                        for j in range(2):
                            so = so2 + j
                            nc.tensor.matmul(
                                ps_ctx[hh][:], v_sb[:, so, h, :],
                                exs[hh][:, j * T:(j + 1) * T],
                                start=(so == 0), stop=(so == SC - 1))

                def attn_evict(hp, hh, ps_ctx, ctxT, psBC):
                    base = hh * HD
                    rden_f = tmpp.tile([1, T], f32, tag="rden_f")
                    act_reciprocal(rden_f[:], ps_ctx[hh][HD:HD + 1, :])
                    rden_bf = tmpp.tile([1, T], bf16, tag="rden_bf")
                    nc.vector.tensor_copy(rden_bf[:], rden_f[:])
                    ps_bc = psBC.tile([HD, T], f32, tag="psBC")
                    nc.tensor.matmul(ps_bc[:], ones64_bf[:],
                                     rden_bf[:], start=True, stop=True)
                    bc_sb = tmpp.tile([HD, T], f32, tag="bc_sb")
                    nc.vector.tensor_copy(bc_sb[:], ps_bc[:])
                    nc.vector.tensor_tensor(
                        ctxT[base:base + HD, hp, :],
                        ps_ctx[hh][0:HD, :], bc_sb[:], ALU.mult)

                with (
                    tc.tile_pool(name="psK", bufs=1, space="PSUM") as psK,
                    tc.tile_pool(name="psSC", bufs=2, space="PSUM") as psSC,
                    tc.tile_pool(name="psCTX", bufs=2, space="PSUM") as psCTX,
                    tc.tile_pool(name="psBC", bufs=1, space="PSUM") as psBC,
                ):
                    for hp in range(DC):
                        ktch = attn_kproj(hp, kin, psK)
                        ps_ctx = [psCTX.tile([HD + 1, T], f32, tag="psCTX",
                                            name=f"ps_ctx{i}")
                                  for i in range(2)]
                        for so2 in range(0, SC, 2):
                            attn_scores_ctx(hp, so2, ktch, qt, v_sb,
                                            ps_ctx, psSC)
                        for hh in range(2):
                            attn_evict(hp, hh, ps_ctx, ctxT, psBC)

                # ================= phase C: out_proj + residual ==========
                xT = act.tile([P, DC, T], f32r, tag="act")
                with tc.tile_pool(name="psC", bufs=2, space="PSUM") as psC:
                    for mo in range(DC):
                        wo_sl = wstream.tile([P, DC, P], bf16, tag="wo_sl")
                        nc.gpsimd.dma_start(wo_sl[:], wo_d.ap().rearrange(
                            "(k p) o -> p k o", p=P)[:, :, mo * P:(mo + 1) * P])
                        ps = psC.tile([P, T], f32, tag="psC")
                        for ki in range(DC):
                            nc.tensor.matmul(ps[:], wo_sl[:, ki, :],
                                             ctxT[:, ki, :],
                                             start=(ki == 0), stop=(ki == DC - 1))
                        nc.vector.scalar_tensor_tensor(
                            xT[:, mo, :], ps[:], bo_pc[:, mo:mo + 1],
                            F(qin[:, mo, :]), op0=ALU.add, op1=ALU.add)

                # ================= phase D: LN2 ==========================
                hT = act.tile([P, DC, T], f32r, tag="act")
                with (
                    tc.tile_pool(name="psST", bufs=1, space="PSUM") as psST,
                    tc.tile_pool(name="psLB", bufs=2, space="PSUM") as psLB,
                ):
                    ln_pass(xT, hT, ln2g, ln2bn, (psST, psLB))

                # ================= phase E: MLP (fused fc1->gelu->fc2) ===
                x2T = act.tile([P, DC, T], f32r, tag="act")
                with (
                    tc.tile_pool(name="psF1", bufs=2, space="PSUM") as psF1,
                    tc.tile_pool(name="psF2", bufs=6, space="PSUM") as psF2,
                ):
                    ps_f2 = [psF2.tile([P, T], f32, tag="psF2", name=f"ps_f2_{i}")
                             for i in range(DC)]
                    for fo in range(FFC):
                        f1_sl = fstream.tile([P, DC, P], f32r, tag="f1_sl")
                        nc.sync.dma_start(f1_sl[:], fc1_d.ap().rearrange(
                            "(k p) f -> p k f", p=P)[:, :, fo * P:(fo + 1) * P])
                        f2_sl = fstream.tile([P, D], f32r, tag="f2_sl")
                        nc.sync.dma_start(f2_sl[:], fc2_d.ap().rearrange(
                            "(ko p) o -> p ko o", p=P)[:, fo, :])
                        ps1 = psF1.tile([P, T], f32, tag="psF1")
                        for ki in range(DC):
                            nc.tensor.matmul(ps1[:], f1_sl[:, ki, :],
                                             hT[:, ki, :],
                                             start=(ki == 0), stop=(ki == DC - 1))
                        mch = mchunkp.tile([P, T], f32r, tag="mch")
                        nc.scalar.activation(mch[:], ps1[:], AF.Gelu,
                                             bias=f1b_pc[:, fo:fo + 1])
                        for mo in range(DC):
                            nc.tensor.matmul(
                                ps_f2[mo][:], f2_sl[:, mo * P:(mo + 1) * P],
                                mch[:],
                                start=(fo == 0), stop=(fo == FFC - 1))
                    for mo in range(DC):
                        nc.vector.scalar_tensor_tensor(
                            x2T[:, mo, :], ps_f2[mo][:], f2b_pc[:, mo:mo + 1],
                            F(xT[:, mo, :]), op0=ALU.add, op1=ALU.add)

                # ================= phase F: LNp + store ==================
                outT = act.tile([P, DC, T], f32, tag="act")
                with (
                    tc.tile_pool(name="psST2", bufs=1, space="PSUM") as psST2,
                    tc.tile_pool(name="psLB2", bufs=2, space="PSUM") as psLB2,
                ):
                    ln_pass(x2T, outT, lnpg, lnpbn, (psST2, psLB2))
                nc.sync.dma_start(
                    out_d.ap()[b].rearrange("(c p) t -> p c t", p=P), outT[:])

    nc.compile()
    return nc


def _get_nc(use_bv: bool):
    key = ("nc", use_bv)
    if key not in _cached:
        _cached[key] = _build(use_bv)
    return _cached[key]


def kernel(query, key, wq, bq, wk, bk, wv, bv, wo, bo,
           ln2_g, ln2_b, fc1_w, fc1_b, fc2_w, fc2_b, lnp_g, lnp_b):
    from concourse.bass_utils import run_bass_kernel_spmd

    f = np.float32
    c = np.ascontiguousarray
    query = np.asarray(query, f)
    key = np.asarray(key, f)
    use_bv = bool(np.any(np.asarray(bv)))
    nc = _get_nc(use_bv)

    shared = {
        "wq_t": c(np.asarray(wq, f).T * np.float32(SCALE)),
        "wk_t": c(np.asarray(wk, f).T),
        "wv_t": c(np.asarray(wv, f).T),
        "wo_t": c(np.asarray(wo, f).T),
        "fc1_t": c(np.asarray(fc1_w, f).T),
        "fc2_t": c(np.asarray(fc2_w, f).T),
        "bqv": c(np.asarray(bq, f) * np.float32(SCALE)),
        "bkv": c(np.asarray(bk, f)),
        "bvv": c(np.asarray(bv, f).reshape(1, D)),
        "bov": c(np.asarray(bo, f)),
        "f1b": c(np.asarray(fc1_b, f)),
        "f2b": c(np.asarray(fc2_b, f)),
        "ln2g": c(np.asarray(ln2_g, f).reshape(1, D)),
        "ln2bn": c(-np.asarray(ln2_b, f).reshape(1, D)),
        "lnpg": c(np.asarray(lnp_g, f).reshape(1, D)),
        "lnpbn": c(-np.asarray(lnp_b, f).reshape(1, D)),
        "ones_col": np.ones((P, 1), f),
        "ones_row": np.ones((1, T), f),
    }
    in_maps = []
    for core in range(NCORES):
        sl = slice(core * BPC, (core + 1) * BPC)
        m = dict(shared)
        m["qT"] = c(query[sl].transpose(0, 2, 1))
        m["kT"] = c(key[sl].transpose(0, 2, 1))
        in_maps.append(m)

    res = run_bass_kernel_spmd(nc, in_maps, core_ids=list(range(NCORES)))
    kernel._last_result = res
    out = np.concatenate([r["out"] for r in res.results], axis=0)
    return c(out.transpose(0, 2, 1))



# revision 16
# speedup vs baseline: 1.0353x; 1.0353x over previous
"""GroupViT cross-attention layer on 8 TRN2 NeuronCores.

Strategy: pure data-parallel over batch (16 batches -> 2 per core, zero
collectives). Feature-major ("transposed") layout on chip: activations
stored [feature(partition), token(free)], weights host-transposed to
[d_in, d_out] so every matmul contracts over the partition dim.

Perf notes (v2):
- All big stationary operands are bf16 with exactly 128 columns so the
  compiler's fast-weight-load path hides LDWEIGHTS under the matmul
  stream (measured: V-proj 195ns/MM vs 330-390 for f32r stationaries).
- K-projection bias dropped entirely: it adds q_t.bk to every score in
  a softmax row (constant over s), so softmax is invariant to it.
- Score matmuls for the two heads of a partition chunk run on disjoint
  64-row PE tile_positions -> concurrent streaming (row packing).
- Softmax denominators come free from a ones column appended to V; the
  reciprocals for a whole batch are done in ONE Activation instruction
  after all EXPs (avoids exp<->reciprocal table thrashing), and the
  per-head broadcast across partitions runs on the idle GpSimd engine
  (partition_broadcast) instead of K=1 matmuls on the PE.
- LayerNorm over the partition (feature) dim: sums via ones-column
  matmuls; mean/rsqrt rows broadcast across partitions on GpSimd; the
  (x - mu)*rs apply is split between Vector and GpSimd.
- kin double-buffered so batch 1's K/V DMA overlaps batch 0 compute;
  big weights are DMA'd once (bf16 from host, halves HBM traffic).
"""

import numpy as np

B, T, S, D, H, HD, FF = 16, 512, 2048, 768, 12, 64, 3072
NCORES = 8
BPC = B // NCORES      # batches per core
P = 128
DC = D // P            # 6 feature chunks
SC = S // P            # 16 key-token chunks
FFC = FF // P          # 24
EPS = 1e-5
SCALE = HD ** -0.5

_cached = {}


def _build(use_bv: bool, ln2_triv: bool, lnp_triv: bool):
    import concourse.bacc as bacc
    import concourse.tile as tile
    import concourse.mybir as mybir

    f32 = mybir.dt.float32
    f32r = mybir.dt.float32r
    bf16 = mybir.dt.bfloat16
    AF = mybir.ActivationFunctionType
    ALU = mybir.AluOpType

    nc = bacc.Bacc("TRN2", target_bir_lowering=False, debug=False,
                   num_devices=NCORES)

    # ---- DRAM I/O (per-core shapes) ----
    qT_d = nc.dram_tensor("qT", [BPC, D, T], f32r, kind="ExternalInput")
    kT_d = nc.dram_tensor("kT", [BPC, D, S], bf16, kind="ExternalInput")
    wq_d = nc.dram_tensor("wq_t", [D, D], f32r, kind="ExternalInput")
    wk_d = nc.dram_tensor("wk_t", [D, D], bf16, kind="ExternalInput")
    wv_d = nc.dram_tensor("wv_t", [D, D], bf16, kind="ExternalInput")
    wo_d = nc.dram_tensor("wo_t", [D, D], bf16, kind="ExternalInput")
    fc1_d = nc.dram_tensor("fc1_t", [D, FF], bf16, kind="ExternalInput")
    fc2_d = nc.dram_tensor("fc2_t", [FF, D], bf16, kind="ExternalInput")
    bq_d = nc.dram_tensor("bqv", [D], f32, kind="ExternalInput")
    bv_d = nc.dram_tensor("bvv", [1, D], f32, kind="ExternalInput")
    bo_d = nc.dram_tensor("bov", [D], f32, kind="ExternalInput")
    f1b_d = nc.dram_tensor("f1b", [FF], f32, kind="ExternalInput")
    f2b_d = nc.dram_tensor("f2b", [D], f32, kind="ExternalInput")
    ln2g_d = nc.dram_tensor("ln2g", [D], f32, kind="ExternalInput")
    ln2b_d = nc.dram_tensor("ln2b", [D], f32, kind="ExternalInput")
    lnpg_d = nc.dram_tensor("lnpg", [D], f32, kind="ExternalInput")
    lnpb_d = nc.dram_tensor("lnpb", [D], f32, kind="ExternalInput")
    ones_col_d = nc.dram_tensor("ones_col", [P, 1], f32r, kind="ExternalInput")
    out_d = nc.dram_tensor("out", [BPC, D, T], f32, kind="ExternalOutput")

    def F(ap):
        return ap.bitcast(f32)

    def act_reciprocal(out, in_):
        eng = nc.scalar
        ins = [eng.lower_ap(in_)]
        for v in (0.0, 1.0, 0.0):
            ins.append(mybir.ImmediateValue(dtype=f32, value=v))
        return eng.add_instruction(mybir.InstActivation(
            name=nc.get_next_instruction_name(),
            func=AF.Reciprocal, ins=ins, outs=[eng.lower_ap(out)]))

    with tile.TileContext(nc) as tc:
        with (
            tc.tile_pool(name="act", bufs=3) as act,
            tc.tile_pool(name="bigk", bufs=1) as bigk,
            tc.tile_pool(name="vpool", bufs=1) as vpool,
            tc.tile_pool(name="qtp", bufs=1) as qtp,
            tc.tile_pool(name="ktc", bufs=2) as ktc,
            tc.tile_pool(name="wstream", bufs=2) as wstream,
            tc.tile_pool(name="wbig", bufs=1) as wbig,
            tc.tile_pool(name="fstream", bufs=2) as fstream,
            tc.tile_pool(name="expp", bufs=3) as expp,
            tc.tile_pool(name="mchunk", bufs=3) as mchunkp,
            tc.tile_pool(name="tmp", bufs=3) as tmpp,
            tc.tile_pool(name="bcp", bufs=2) as bcp,
            tc.tile_pool(name="small", bufs=1) as small,
        ):
            # ---- persistent small tiles ----
            ones_col = small.tile([P, 1], f32r, tag="ones_col")
            nc.sync.dma_start(ones_col[:], ones_col_d.ap())
            onesc_f = small.tile([P, 1], f32, tag="onesc_f")
            nc.vector.memset(onesc_f[:], 1.0)
            eps_t = small.tile([1, 1], f32, tag="eps")
            nc.vector.memset(eps_t[:], EPS)

            bq_pc = small.tile([P, DC], f32, tag="bq_pc")
            nc.sync.dma_start(bq_pc[:], bq_d.ap().rearrange("(c p) -> p c", p=P))
            bo_pc = small.tile([P, DC], f32, tag="bo_pc")
            nc.sync.dma_start(bo_pc[:], bo_d.ap().rearrange("(c p) -> p c", p=P))
            f1b_pc = small.tile([P, FFC], f32, tag="f1b_pc")
            nc.sync.dma_start(f1b_pc[:], f1b_d.ap().rearrange("(c p) -> p c", p=P))
            f2b_pc = small.tile([P, DC], f32, tag="f2b_pc")
            nc.sync.dma_start(f2b_pc[:], f2b_d.ap().rearrange("(c p) -> p c", p=P))

            ln2g_pc = ln2b_pc = lnpg_pc = lnpb_pc = None
            if not ln2_triv:
                ln2g_pc = small.tile([P, DC], f32, tag="ln2g_pc")
                nc.sync.dma_start(ln2g_pc[:],
                                  ln2g_d.ap().rearrange("(c p) -> p c", p=P))
                ln2b_pc = small.tile([P, DC], f32, tag="ln2b_pc")
                nc.sync.dma_start(ln2b_pc[:],
                                  ln2b_d.ap().rearrange("(c p) -> p c", p=P))
            if not lnp_triv:
                lnpg_pc = small.tile([P, DC], f32, tag="lnpg_pc")
                nc.sync.dma_start(lnpg_pc[:],
                                  lnpg_d.ap().rearrange("(c p) -> p c", p=P))
                lnpb_pc = small.tile([P, DC], f32, tag="lnpb_pc")
                nc.sync.dma_start(lnpb_pc[:],
                                  lnpb_d.ap().rearrange("(c p) -> p c", p=P))

            bv_row = None
            if use_bv:
                bv_row = small.tile([1, D], f32, tag="bv_row")
                nc.sync.dma_start(bv_row[:], bv_d.ap())

            # ---- big weights, loaded once (bf16, 128-col stationaries) ----
            wk_sb = wbig.tile([P, DC, D], bf16, tag="wk")
            nc.gpsimd.dma_start(wk_sb[:], wk_d.ap().rearrange(
                "(k p) o -> p k o", p=P))
            wv_sb = wbig.tile([P, DC, D], bf16, tag="wv")
            nc.gpsimd.dma_start(wv_sb[:], wv_d.ap().rearrange(
                "(k p) o -> p k o", p=P))
            wo_sb = wbig.tile([P, DC, D], bf16, tag="wo")
            nc.gpsimd.dma_start(wo_sb[:], wo_d.ap().rearrange(
                "(k p) o -> p k o", p=P))

            def ln_pass(xsrc, dst, g_pc, b_pc, trivial, psST):
                """LayerNorm over the partition(feature) dim:
                xsrc [P, DC, T] f32r -> dst [P, DC, T] (dst dtype as made)."""
                psum_mu = psST.tile([1, T], f32, tag="st_mu")
                psum_sq = psST.tile([1, T], f32, tag="st_sq")
                for c in range(DC):
                    nc.tensor.matmul(psum_mu[:], ones_col[:], xsrc[:, c, :],
                                     start=(c == 0), stop=(c == DC - 1))
                sqt = []
                for c in range(DC):
                    sq = tmpp.tile([P, T], f32r, tag="lnsq")
                    nc.vector.tensor_tensor(sq[:], F(xsrc[:, c, :]),
                                            F(xsrc[:, c, :]), ALU.mult)
                    sqt.append(sq)
                for c in range(DC):
                    nc.tensor.matmul(psum_sq[:], ones_col[:], sqt[c][:],
                                     start=(c == 0), stop=(c == DC - 1))
                mu_f = small.tile([1, T], f32, tag="ln_mu")
                nc.vector.tensor_scalar_mul(mu_f[:], psum_mu[:], 1.0 / D)
                mu2_f = small.tile([1, T], f32, tag="ln_mu2")
                nc.vector.tensor_tensor(mu2_f[:], mu_f[:], mu_f[:], ALU.mult)
                var_f = small.tile([1, T], f32, tag="ln_var")
                nc.vector.scalar_tensor_tensor(
                    var_f[:], psum_sq[:], 1.0 / D, mu2_f[:],
                    op0=ALU.mult, op1=ALU.subtract)
                rs_f = small.tile([1, T], f32, tag="ln_rs")
                nc.scalar.activation(rs_f[:], var_f[:], AF.Abs_reciprocal_sqrt,
                                     bias=eps_t[:])
                mrs_f = small.tile([1, T], f32, tag="ln_mrs")
                nc.vector.tensor_tensor(mrs_f[:], mu_f[:], rs_f[:], ALU.mult)
                rs_bc = bcp.tile([P, T], f32, tag="ln_rs_bc")
                nc.gpsimd.partition_broadcast(rs_bc[:], rs_f[:])
                mrs_bc = bcp.tile([P, T], f32, tag="ln_mrs_bc")
                nc.gpsimd.partition_broadcast(mrs_bc[:], mrs_f[:])
                for c in range(DC):
                    eng = nc.vector if c % 2 == 0 else nc.gpsimd
                    if trivial:
                        tmp = tmpp.tile([P, T], f32, tag="ln_tmp")
                        eng.tensor_tensor(tmp[:], F(xsrc[:, c, :]), rs_bc[:],
                                          ALU.mult)
                        eng.tensor_tensor(dst[:, c, :], tmp[:], mrs_bc[:],
                                          ALU.subtract)
                    else:
                        tmp = tmpp.tile([P, T], f32, tag="ln_tmp")
                        eng.tensor_tensor(tmp[:], F(xsrc[:, c, :]), rs_bc[:],
                                          ALU.mult)
                        tmp2 = tmpp.tile([P, T], f32, tag="ln_tmp2")
                        eng.tensor_tensor(tmp2[:], tmp[:], mrs_bc[:],
                                          ALU.subtract)
                        eng.tensor_scalar(dst[:, c, :], tmp2[:],
                                          g_pc[:, c:c + 1], b_pc[:, c:c + 1],
                                          op0=ALU.mult, op1=ALU.add)

            for b in range(BPC):
                # ================= phase A: load + Q/V projections ======
                qin = act.tile([P, DC, T], f32r, tag="act")
                nc.sync.dma_start(qin[:], qT_d.ap()[b].rearrange(
                    "(c p) t -> p c t", p=P))
                kin = bigk.tile([P, DC, S], bf16, tag="kin")
                nc.gpsimd.dma_start(kin[:], kT_d.ap()[b].rearrange(
                    "(c p) s -> p c s", p=P))

                qt = qtp.tile([P, DC, T], bf16, tag="qt")
                with tc.tile_pool(name="psA", bufs=2, space="PSUM") as psA:
                    for mo in range(DC):
                        wq_sl = wstream.tile([P, DC, P], f32r, tag="wq_sl")
                        nc.sync.dma_start(wq_sl[:], wq_d.ap().rearrange(
                            "(k p) o -> p k o", p=P)[:, :, mo * P:(mo + 1) * P])
                        ps = psA.tile([P, T], f32, tag="psA")
                        for ki in range(DC):
                            nc.tensor.matmul(ps[:], wq_sl[:, ki, :],
                                             qin[:, ki, :],
                                             start=(ki == 0), stop=(ki == DC - 1))
                        nc.vector.tensor_scalar_add(qt[:, mo, :], ps[:],
                                                    bq_pc[:, mo:mo + 1])

                    v_sb = vpool.tile([P, SC, H, HD + 1], bf16, tag="v")
                    nc.vector.tensor_copy(
                        v_sb[:, :, :, HD:HD + 1],
                        onesc_f[:].to_broadcast([P, SC, H, 1]))
                    bv_bc = None
                    if use_bv:
                        bv_bc = small.tile([P, D], f32, tag="bv_bc")
                        for half in range(2):
                            nc.gpsimd.partition_broadcast(
                                bv_bc[:, half * 384:(half + 1) * 384],
                                bv_row[:, half * 384:(half + 1) * 384])
                    for so in range(SC):
                        for half in range(2):
                            ps = psA.tile([P, 384], f32, tag="psA")
                            for ki in range(DC):
                                nc.tensor.matmul(
                                    ps[:],
                                    kin[:, ki, so * P:(so + 1) * P],
                                    wv_sb[:, ki, half * 384:(half + 1) * 384],
                                    start=(ki == 0), stop=(ki == DC - 1))
                            dstv = v_sb[:, so, half * 6:(half + 1) * 6, 0:HD]
                            if use_bv:
                                nc.vector.tensor_tensor(
                                    dstv, ps[:],
                                    bv_bc[:, half * 384:(half + 1) * 384],
                                    ALU.add)
                            else:
                                nc.vector.tensor_copy(dstv, ps[:])

                # ================= phase B: attention ====================
                ctxT = act.tile([P, DC, T], bf16, tag="act")
                # denominators staged free-dim-major on partition 0 (engine
                # APs must start on 32-aligned partitions)
                den_sb = small.tile([1, H, T], bf16, tag="den_sb")

                def attn_kproj(hp, kin, psK):
                    ktch = ktc.tile([P, S], bf16, tag="ktc")
                    for no in range(4):
                        ps = psK.tile([P, T], f32, tag="psK")
                        for ki in range(DC):
                            nc.tensor.matmul(
                                ps[:], wk_sb[:, ki, hp * P:(hp + 1) * P],
                                kin[:, ki, no * T:(no + 1) * T],
                                start=(ki == 0), stop=(ki == DC - 1))
                        nc.vector.tensor_copy(
                            ktch[:, no * T:(no + 1) * T], ps[:])
                    return ktch

                def attn_scores_ctx(hp, so2, ktch, qt, v_sb, ps_ctx, psSC):
                    scs = [psSC.tile([P, 2 * T], f32, tag="psSC",
                                     name=f"ps_sc{hh}")
                           for hh in range(2)]
                    # hh-interleaved scores: the two heads stream on
                    # disjoint 64-row PE tiles concurrently
                    for j in range(2):
                        so = so2 + j
                        for hh in range(2):
                            base = hh * HD
                            nc.tensor.matmul(
                                scs[hh][:, j * T:(j + 1) * T],
                                ktch[base:base + HD, so * P:(so + 1) * P],
                                qt[base:base + HD, hp, :],
                                start=True, stop=True,
                                tile_position=(base, 0))
                    exs = []
                    for hh in range(2):
                        ex = expp.tile([P, 2 * T], bf16, tag="exp",
                                       name=f"ex{hh}")
                        nc.scalar.activation(ex[:], scs[hh][:], AF.Exp)
                        exs.append(ex)
                    for j in range(2):
                        so = so2 + j
                        for hh in range(2):
                            h = 2 * hp + hh
                            nc.tensor.matmul(
                                ps_ctx[hh][:], v_sb[:, so, h, :],
                                exs[hh][:, j * T:(j + 1) * T],
                                start=(so == 0), stop=(so == SC - 1))

                def attn_evict(hp, hh, ps_ctx, ctxT, den_sb):
                    h = 2 * hp + hh
                    base = hh * HD
                    nc.vector.tensor_copy(
                        ctxT[base:base + HD, hp, :], ps_ctx[hh][0:HD, :])
                    # ACT Copy: allows partition remap, and 'copy' is in
                    # every activation table set (no table load)
                    nc.scalar.activation(
                        den_sb[0:1, h, :], ps_ctx[hh][HD:HD + 1, :], AF.Copy)

                with (
                    tc.tile_pool(name="psK", bufs=2, space="PSUM") as psK,
                    tc.tile_pool(name="psSC", bufs=2, space="PSUM") as psSC,
                    tc.tile_pool(name="psCTX", bufs=2, space="PSUM") as psCTX,
                ):
                    for hp in range(DC):
                        # K-projection for this head pair (no bias: softmax
                        # is invariant to the K bias)
                        ktch = attn_kproj(hp, kin, psK)
                        ps_ctx = [psCTX.tile([HD + 1, T], f32, tag="psCTX",
                                             name=f"ps_ctx{i}")
                                  for i in range(2)]
                        for so2 in range(0, SC, 2):
                            attn_scores_ctx(hp, so2, ktch, qt, v_sb,
                                            ps_ctx, psSC)
                        for hh in range(2):
                            attn_evict(hp, hh, ps_ctx, ctxT, den_sb)

                # batch-wide softmax normalization: reciprocals emitted after
                # all EXPs (one table load), broadcast on GpSimd
                for h in range(H):
                    act_reciprocal(den_sb[0:1, h, :], den_sb[0:1, h, :])
                for h in range(H):
                    hp, hh = divmod(h, 2)
                    base = hh * HD
                    bc = bcp.tile([P, T], bf16, tag="den_bc")
                    nc.gpsimd.partition_broadcast(bc[:], den_sb[0:1, h, :])
                    nc.vector.tensor_tensor(
                        ctxT[base:base + HD, hp, :],
                        ctxT[base:base + HD, hp, :],
                        bc[base:base + HD, :], ALU.mult)

                # ================= phase C: out_proj + residual ==========
                xT = act.tile([P, DC, T], f32r, tag="act")
                with tc.tile_pool(name="psC", bufs=2, space="PSUM") as psC:
                    for mo in range(DC):
                        ps = psC.tile([P, T], f32, tag="psC")
                        for ki in range(DC):
                            nc.tensor.matmul(ps[:],
                                             wo_sb[:, ki, mo * P:(mo + 1) * P],
                                             ctxT[:, ki, :],
                                             start=(ki == 0), stop=(ki == DC - 1))
                        nc.vector.scalar_tensor_tensor(
                            xT[:, mo, :], ps[:], bo_pc[:, mo:mo + 1],
                            F(qin[:, mo, :]), op0=ALU.add, op1=ALU.add)

                # ================= phase D: LN2 ==========================
                hT = act.tile([P, DC, T], bf16, tag="act_h", bufs=1)
                with tc.tile_pool(name="psST", bufs=1, space="PSUM") as psST:
                    ln_pass(xT, hT, ln2g_pc, ln2b_pc, ln2_triv, psST)

                # ================= phase E: MLP (fused fc1->gelu->fc2) ===
                x2T = act.tile([P, DC, T], f32r, tag="act")
                with (
                    tc.tile_pool(name="psF1", bufs=2, space="PSUM") as psF1,
                    tc.tile_pool(name="psF2", bufs=6, space="PSUM") as psF2,
                ):
                    ps_f2 = [psF2.tile([P, T], f32, tag="psF2", name=f"ps_f2_{i}")
                             for i in range(DC)]
                    for fo in range(FFC):
                        f1_sl = fstream.tile([P, DC, P], bf16, tag="f1_sl")
                        nc.sync.dma_start(f1_sl[:], fc1_d.ap().rearrange(
                            "(k p) f -> p k f", p=P)[:, :, fo * P:(fo + 1) * P])
                        f2_sl = fstream.tile([P, D], bf16, tag="f2_sl")
                        nc.sync.dma_start(f2_sl[:], fc2_d.ap().rearrange(
                            "(ko p) o -> p ko o", p=P)[:, fo, :])
                        ps1 = psF1.tile([P, T], f32, tag="psF1")
                        for ki in range(DC):
                            nc.tensor.matmul(ps1[:], f1_sl[:, ki, :],
                                             hT[:, ki, :],
                                             start=(ki == 0), stop=(ki == DC - 1))
                        mch = mchunkp.tile([P, T], bf16, tag="mch")
                        nc.scalar.activation(mch[:], ps1[:], AF.Gelu,
                                             bias=f1b_pc[:, fo:fo + 1])
                        for mo in range(DC):
                            nc.tensor.matmul(
                                ps_f2[mo][:], f2_sl[:, mo * P:(mo + 1) * P],
                                mch[:],
                                start=(fo == 0), stop=(fo == FFC - 1))
                    for mo in range(DC):
                        nc.vector.scalar_tensor_tensor(
                            x2T[:, mo, :], ps_f2[mo][:], f2b_pc[:, mo:mo + 1],
                            F(xT[:, mo, :]), op0=ALU.add, op1=ALU.add)

                # ================= phase F: LNp + store ==================
                outT = act.tile([P, DC, T], f32, tag="act")
                with tc.tile_pool(name="psST2", bufs=1, space="PSUM") as psST2:
                    ln_pass(x2T, outT, lnpg_pc, lnpb_pc, lnp_triv, psST2)
                nc.sync.dma_start(
                    out_d.ap()[b].rearrange("(c p) t -> p c t", p=P), outT[:])

    nc.compile()
    return nc


def _get_nc(use_bv: bool, ln2_triv: bool, lnp_triv: bool):
    key = ("nc", use_bv, ln2_triv, lnp_triv)
    if key not in _cached:
        _cached[key] = _build(use_bv, ln2_triv, lnp_triv)
    return _cached[key]


def kernel(query, key, wq, bq, wk, bk, wv, bv, wo, bo,
           ln2_g, ln2_b, fc1_w, fc1_b, fc2_w, fc2_b, lnp_g, lnp_b):
    from concourse.bass_utils import run_bass_kernel_spmd
    import ml_dtypes

    f = np.float32
    bf = ml_dtypes.bfloat16
    c = np.ascontiguousarray

    def cbf(a):
        return np.ascontiguousarray(np.asarray(a, f).astype(bf))

    query = np.asarray(query, f)
    key = np.asarray(key, f)
    use_bv = bool(np.any(np.asarray(bv)))
    ln2_triv = (not np.any(np.asarray(ln2_b))) and bool(
        np.all(np.asarray(ln2_g) == 1.0))
    lnp_triv = (not np.any(np.asarray(lnp_b))) and bool(
        np.all(np.asarray(lnp_g) == 1.0))
    nc = _get_nc(use_bv, ln2_triv, lnp_triv)

    shared = {
        "wq_t": c(np.asarray(wq, f).T * np.float32(SCALE)),
        "wk_t": cbf(np.asarray(wk, f).T),
        "wv_t": cbf(np.asarray(wv, f).T),
        "wo_t": cbf(np.asarray(wo, f).T),
        "fc1_t": cbf(np.asarray(fc1_w, f).T),
        "fc2_t": cbf(np.asarray(fc2_w, f).T),
        "bqv": c(np.asarray(bq, f) * np.float32(SCALE)),
        "bvv": c(np.asarray(bv, f).reshape(1, D)),
        "bov": c(np.asarray(bo, f)),
        "f1b": c(np.asarray(fc1_b, f)),
        "f2b": c(np.asarray(fc2_b, f)),
        "ln2g": c(np.asarray(ln2_g, f)),
        "ln2b": c(np.asarray(ln2_b, f)),
        "lnpg": c(np.asarray(lnp_g, f)),
        "lnpb": c(np.asarray(lnp_b, f)),
        "ones_col": np.ones((P, 1), f),
    }
    in_maps = []
    for core in range(NCORES):
        sl = slice(core * BPC, (core + 1) * BPC)
        m = dict(shared)
        m["qT"] = c(query[sl].transpose(0, 2, 1))
        m["kT"] = np.ascontiguousarray(
            key[sl].transpose(0, 2, 1).astype(bf))
        in_maps.append(m)

    res = run_bass_kernel_spmd(nc, in_maps, core_ids=list(range(NCORES)))
    kernel._last_result = res
    out = np.concatenate([r["out"] for r in res.results], axis=0)
    return c(out.transpose(0, 2, 1))


# revision 35
# speedup vs baseline: 1.3089x; 1.2642x over previous
"""GroupViT cross-attention layer on 8 TRN2 NeuronCores.

Strategy: pure data-parallel over batch (16 batches -> 2 per core, zero
collectives). Feature-major ("transposed") layout on chip: activations
stored [feature(partition), token(free)], weights host-transposed to
[d_in, d_out] so every matmul contracts over the partition dim.

Perf notes (v2):
- All big stationary operands are bf16 with exactly 128 columns so the
  compiler's fast-weight-load path hides LDWEIGHTS under the matmul
  stream (measured: V-proj 195ns/MM vs 330-390 for f32r stationaries).
- K-projection bias dropped entirely: it adds q_t.bk to every score in
  a softmax row (constant over s), so softmax is invariant to it.
- Score matmuls for the two heads of a partition chunk run on disjoint
  64-row PE tile_positions -> concurrent streaming (row packing).
- Softmax denominators come free from a ones column appended to V; the
  reciprocals for a whole batch are done in ONE Activation instruction
  after all EXPs (avoids exp<->reciprocal table thrashing), and the
  per-head broadcast across partitions runs on the idle GpSimd engine
  (partition_broadcast) instead of K=1 matmuls on the PE.
- LayerNorm over the partition (feature) dim: sums via ones-column
  matmuls; mean/rsqrt rows broadcast across partitions on GpSimd; the
  (x - mu)*rs apply is split between Vector and GpSimd.
- kin double-buffered so batch 1's K/V DMA overlaps batch 0 compute;
  big weights are DMA'd once (bf16 from host, halves HBM traffic).
"""

import numpy as np

B, T, S, D, H, HD, FF = 16, 512, 2048, 768, 12, 64, 3072
NCORES = 8
BPC = B // NCORES      # batches per core
P = 128
DC = D // P            # 6 feature chunks
SC = S // P            # 16 key-token chunks
FFC = FF // P          # 24
EPS = 1e-5
SCALE = HD ** -0.5

_cached = {}


def _build(use_bv: bool, ln2_triv: bool, lnp_triv: bool):
    import concourse.bacc as bacc
    import concourse.tile as tile
    import concourse.mybir as mybir

    f32 = mybir.dt.float32
    f32r = mybir.dt.float32r
    bf16 = mybir.dt.bfloat16
    f8 = mybir.dt.float8e4
    AF = mybir.ActivationFunctionType
    ALU = mybir.AluOpType
    DR = mybir.MatmulPerfMode.DoubleRow

    nc = bacc.Bacc("TRN2", target_bir_lowering=False, debug=False,
                   num_devices=NCORES)

    # ---- DRAM I/O (per-core shapes) ----
    qT_d = nc.dram_tensor("qT", [BPC, D, T], f32r, kind="ExternalInput")
    qTb_d = nc.dram_tensor("qTb", [BPC, D, T], bf16, kind="ExternalInput")
    kT_d = nc.dram_tensor("kT", [BPC, D, S], bf16, kind="ExternalInput")
    wq_d = nc.dram_tensor("wq_t", [D, D], bf16, kind="ExternalInput")
    wk_d = nc.dram_tensor("wk_t", [D, D], bf16, kind="ExternalInput")
    wv_d = nc.dram_tensor("wv_t", [D, D], bf16, kind="ExternalInput")
    wo_d = nc.dram_tensor("wo_t", [D, D], bf16, kind="ExternalInput")
    fc1_d = nc.dram_tensor("fc1_t", [D, FF], bf16, kind="ExternalInput")
    fc2_d = nc.dram_tensor("fc2_t", [FF, D], bf16, kind="ExternalInput")
    bq_d = nc.dram_tensor("bqv", [D], f32, kind="ExternalInput")
    bv_d = nc.dram_tensor("bvv", [1, D], f32, kind="ExternalInput")
    bo_d = nc.dram_tensor("bov", [D], f32, kind="ExternalInput")
    f1b_d = nc.dram_tensor("f1b", [FF], f32, kind="ExternalInput")
    f2b_d = nc.dram_tensor("f2b", [D], f32, kind="ExternalInput")
    ln2g_d = nc.dram_tensor("ln2g", [D], f32, kind="ExternalInput")
    ln2b_d = nc.dram_tensor("ln2b", [D], f32, kind="ExternalInput")
    lnpg_d = nc.dram_tensor("lnpg", [D], f32, kind="ExternalInput")
    lnpb_d = nc.dram_tensor("lnpb", [D], f32, kind="ExternalInput")
    ones_col_d = nc.dram_tensor("ones_col", [P, 1], f32r, kind="ExternalInput")
    out_d = nc.dram_tensor("out", [BPC, D, T], f32, kind="ExternalOutput")

    def F(ap):
        return ap.bitcast(f32)

    def act_reciprocal(out, in_):
        eng = nc.scalar
        ins = [eng.lower_ap(in_)]
        for v in (0.0, 1.0, 0.0):
            ins.append(mybir.ImmediateValue(dtype=f32, value=v))
        return eng.add_instruction(mybir.InstActivation(
            name=nc.get_next_instruction_name(),
            func=AF.Reciprocal, ins=ins, outs=[eng.lower_ap(out)]))

    with tile.TileContext(nc) as tc:
        with (
            tc.tile_pool(name="act", bufs=4) as act,
            tc.tile_pool(name="bigk", bufs=1) as bigk,
            tc.tile_pool(name="vpool", bufs=1) as vpool,
            tc.tile_pool(name="qtp", bufs=1) as qtp,
            tc.tile_pool(name="qbp", bufs=2) as qbp,
            tc.tile_pool(name="ktc", bufs=2) as ktc,
            tc.tile_pool(name="wstream", bufs=2) as wstream,
            tc.tile_pool(name="wbig", bufs=1) as wbig,
            tc.tile_pool(name="fstream", bufs=2) as fstream,
            tc.tile_pool(name="expp", bufs=3) as expp,
            tc.tile_pool(name="mchunk", bufs=3) as mchunkp,
            tc.tile_pool(name="tmp", bufs=3) as tmpp,
            tc.tile_pool(name="bcp", bufs=2) as bcp,
            tc.tile_pool(name="small", bufs=1) as small,
        ):
            # ---- persistent small tiles ----
            ones_col = small.tile([P, 1], f32r, tag="ones_col")
            nc.sync.dma_start(ones_col[:], ones_col_d.ap())
            onesc_f = small.tile([P, 1], f32, tag="onesc_f")
            nc.vector.memset(onesc_f[:], 1.0)
            ones64_bf = small.tile([1, HD], bf16, tag="ones64")
            nc.vector.memset(ones64_bf[:], 1.0)
            eps_t = small.tile([1, 1], f32, tag="eps")
            nc.vector.memset(eps_t[:], EPS)

            bq_pc = small.tile([P, DC], f32, tag="bq_pc")
            nc.sync.dma_start(bq_pc[:], bq_d.ap().rearrange("(c p) -> p c", p=P))
            bo_pc = small.tile([P, DC], f32, tag="bo_pc")
            nc.sync.dma_start(bo_pc[:], bo_d.ap().rearrange("(c p) -> p c", p=P))
            f1b_pc = small.tile([P, FFC], f32, tag="f1b_pc")
            nc.sync.dma_start(f1b_pc[:], f1b_d.ap().rearrange("(c p) -> p c", p=P))
            f2b_pc = small.tile([P, DC], f32, tag="f2b_pc")
            nc.sync.dma_start(f2b_pc[:], f2b_d.ap().rearrange("(c p) -> p c", p=P))

            ln2g_pc = ln2b_pc = lnpg_pc = lnpb_pc = None
            if not ln2_triv:
                ln2g_pc = small.tile([P, DC], f32, tag="ln2g_pc")
                nc.sync.dma_start(ln2g_pc[:],
                                  ln2g_d.ap().rearrange("(c p) -> p c", p=P))
                ln2b_pc = small.tile([P, DC], f32, tag="ln2b_pc")
                nc.sync.dma_start(ln2b_pc[:],
                                  ln2b_d.ap().rearrange("(c p) -> p c", p=P))
            if not lnp_triv:
                lnpg_pc = small.tile([P, DC], f32, tag="lnpg_pc")
                nc.sync.dma_start(lnpg_pc[:],
                                  lnpg_d.ap().rearrange("(c p) -> p c", p=P))
                lnpb_pc = small.tile([P, DC], f32, tag="lnpb_pc")
                nc.sync.dma_start(lnpb_pc[:],
                                  lnpb_d.ap().rearrange("(c p) -> p c", p=P))

            bv_row = None
            if use_bv:
                bv_row = small.tile([1, D], f32, tag="bv_row")
                nc.sync.dma_start(bv_row[:], bv_d.ap())

            # Block-diagonal qt: for head pair hp, plane hh holds that
            # head's 64 q-rows in its own partition half and ZEROS in the
            # other half. Scores then use the full 128-partition ktch as
            # stationary (128 cols -> fast-weight-load) with K=128; the
            # zero rows kill the cross-head terms. Zeros written once.
            qt_bd = qtp.tile([P, DC, 2, T], bf16, tag="qt_bd")
            nc.vector.memset(qt_bd[:], 0.0)

            # ---- big weights, loaded once (bf16, 128-col stationaries) ----
            wk_sb = wbig.tile([P, DC, D], bf16, tag="wk")
            nc.gpsimd.dma_start(wk_sb[:], wk_d.ap().rearrange(
                "(k p) o -> p k o", p=P))
            wv_sb = wbig.tile([P, DC, D], bf16, tag="wv")
            nc.gpsimd.dma_start(wv_sb[:], wv_d.ap().rearrange(
                "(k p) o -> p k o", p=P))
            wo_sb = wbig.tile([P, DC, D], bf16, tag="wo")
            nc.gpsimd.dma_start(wo_sb[:], wo_d.ap().rearrange(
                "(k p) o -> p k o", p=P))

            def ln_stats_chunk(c, xsrc, psum_mu, psum_sq):
                """Stats matmuls for one feature chunk — emitted right after
                the chunk is produced so they interleave with the producing
                phase instead of serializing after it."""
                nc.tensor.matmul(psum_mu[:], ones_col[:], xsrc[:, c, :],
                                 start=(c == 0), stop=(c == DC - 1))
                sq = tmpp.tile([P, T], f32r, tag="lnsq")
                nc.vector.tensor_tensor(sq[:], F(xsrc[:, c, :]),
                                        F(xsrc[:, c, :]), ALU.mult)
                nc.tensor.matmul(psum_sq[:], ones_col[:], sq[:],
                                 start=(c == 0), stop=(c == DC - 1))

            def ln_finish(xsrc, dst, g_pc, b_pc, trivial, psum_mu, psum_sq):
                """mu/var/rsqrt chain + partition broadcast + apply."""
                mu_f = small.tile([1, T], f32, tag="ln_mu")
                nc.vector.tensor_scalar_mul(mu_f[:], psum_mu[:], 1.0 / D)
                mu2_f = small.tile([1, T], f32, tag="ln_mu2")
                nc.vector.tensor_tensor(mu2_f[:], mu_f[:], mu_f[:], ALU.mult)
                var_f = small.tile([1, T], f32, tag="ln_var")
                nc.vector.scalar_tensor_tensor(
                    var_f[:], psum_sq[:], 1.0 / D, mu2_f[:],
                    op0=ALU.mult, op1=ALU.subtract)
                rs_f = small.tile([1, T], f32, tag="ln_rs")
                nc.scalar.activation(rs_f[:], var_f[:], AF.Abs_reciprocal_sqrt,
                                     bias=eps_t[:])
                mrs_f = small.tile([1, T], f32, tag="ln_mrs")
                nc.vector.tensor_tensor(mrs_f[:], mu_f[:], rs_f[:], ALU.mult)
                rs_bc = bcp.tile([P, T], f32, tag="ln_rs_bc")
                nc.gpsimd.partition_broadcast(rs_bc[:], rs_f[:])
                mrs_bc = bcp.tile([P, T], f32, tag="ln_mrs_bc")
                nc.gpsimd.partition_broadcast(mrs_bc[:], mrs_f[:])
                for c in range(DC):
                    eng = nc.vector if c % 2 == 0 else nc.gpsimd
                    if trivial:
                        tmp = tmpp.tile([P, T], f32, tag="ln_tmp")
                        eng.tensor_tensor(tmp[:], F(xsrc[:, c, :]), rs_bc[:],
                                          ALU.mult)
                        eng.tensor_tensor(dst[:, c, :], tmp[:], mrs_bc[:],
                                          ALU.subtract)
                    else:
                        tmp = tmpp.tile([P, T], f32, tag="ln_tmp")
                        eng.tensor_tensor(tmp[:], F(xsrc[:, c, :]), rs_bc[:],
                                          ALU.mult)
                        tmp2 = tmpp.tile([P, T], f32, tag="ln_tmp2")
                        eng.tensor_tensor(tmp2[:], tmp[:], mrs_bc[:],
                                          ALU.subtract)
                        eng.tensor_scalar(dst[:, c, :], tmp2[:],
                                          g_pc[:, c:c + 1], b_pc[:, c:c + 1],
                                          op0=ALU.mult, op1=ALU.add)

            for b in range(BPC):
                # ================= phase A: load + Q/V projections ======
                qin = act.tile([P, DC, T], f32r, tag="act")
                nc.sync.dma_start(qin[:], qT_d.ap()[b].rearrange(
                    "(c p) t -> p c t", p=P))
                qin_bf = qbp.tile([P, DC, T], bf16, tag="qin_bf")
                nc.sync.dma_start(qin_bf[:], qTb_d.ap()[b].rearrange(
                    "(c p) t -> p c t", p=P))
                kin = bigk.tile([P, DC, S], bf16, tag="kin")
                nc.gpsimd.dma_start(kin[:], kT_d.ap()[b].rearrange(
                    "(c p) s -> p c s", p=P))

                with tc.tile_pool(name="psA", bufs=2, space="PSUM") as psA:
                    for mo in range(DC):
                        wq_sl = wstream.tile([P, DC, P], bf16, tag="wq_sl")
                        nc.sync.dma_start(wq_sl[:], wq_d.ap().rearrange(
                            "(k p) o -> p k o", p=P)[:, :, mo * P:(mo + 1) * P])
                        ps = psA.tile([P, T], f32, tag="psA")
                        for ki in range(DC):
                            nc.tensor.matmul(ps[:], wq_sl[:, ki, :],
                                             qin_bf[:, ki, :],
                                             start=(ki == 0), stop=(ki == DC - 1))
                        for hh in range(2):
                            base = hh * HD
                            nc.vector.tensor_scalar_add(
                                qt_bd[base:base + HD, mo, hh, :],
                                ps[base:base + HD, :],
                                bq_pc[base:base + HD, mo:mo + 1])

                    # head stride padded to 68 so the so-plane stride
                    # (H*68 = 816 bytes) is 16-aligned, as DoubleRow
                    # LDWEIGHTS requires
                    v_sb = vpool.tile([P, SC, H, 68], f8, tag="v")
                    nc.vector.tensor_copy(
                        v_sb[:, :, :, HD:HD + 1],
                        onesc_f[:].to_broadcast([P, SC, H, 1]))
                    bv_bc = None
                    if use_bv:
                        bv_bc = small.tile([P, D], f32, tag="bv_bc")
                        for half in range(2):
                            nc.gpsimd.partition_broadcast(
                                bv_bc[:, half * 384:(half + 1) * 384],
                                bv_row[:, half * 384:(half + 1) * 384])
                    for so in range(SC):
                        for half in range(2):
                            ps = psA.tile([P, 384], f32, tag="psA")
                            for ki in range(DC):
                                nc.tensor.matmul(
                                    ps[:],
                                    kin[:, ki, so * P:(so + 1) * P],
                                    wv_sb[:, ki, half * 384:(half + 1) * 384],
                                    start=(ki == 0), stop=(ki == DC - 1))
                            dstv = v_sb[:, so, half * 6:(half + 1) * 6, 0:HD]
                            if use_bv:
                                nc.vector.tensor_tensor(
                                    dstv, ps[:],
                                    bv_bc[:, half * 384:(half + 1) * 384],
                                    ALU.add)
                            else:
                                nc.vector.tensor_copy(dstv, ps[:])

                # ================= phase B: attention ====================
                ctxT = act.tile([P, DC, T], bf16, tag="act")
                # denominators staged free-dim-major on partition 0 (engine
                # APs must start on 32-aligned partitions)
                den_sb = small.tile([1, H, T], bf16, tag="den_sb")

                def attn_kproj(hp, kin, psK):
                    ktch = ktc.tile([P, S], bf16, tag="ktc")
                    for no in range(4):
                        ps = psK.tile([P, T], f32, tag="psK")
                        for ki in range(DC):
                            nc.tensor.matmul(
                                ps[:], wk_sb[:, ki, hp * P:(hp + 1) * P],
                                kin[:, ki, no * T:(no + 1) * T],
                                start=(ki == 0), stop=(ki == DC - 1))
                        nc.vector.tensor_copy(
                            ktch[:, no * T:(no + 1) * T], ps[:])
                    return ktch

                def attn_scores_ctx(hp, so2, ktch, v_sb, ps_ctx, psSC):
                    scs = [psSC.tile([P, 2 * T], f32, tag="psSC",
                                     name=f"ps_sc{hh}")
                           for hh in range(2)]
                    # K=128 scores against block-diagonal qt: full ktch as
                    # 128-col stationary (fast-weight-load hides LDWEIGHTS)
                    for j in range(2):
                        so = so2 + j
                        for hh in range(2):
                            nc.tensor.matmul(
                                scs[hh][:, j * T:(j + 1) * T],
                                ktch[:, so * P:(so + 1) * P],
                                qt_bd[:, hp, hh, :],
                                start=True, stop=True)
                    exs = []
                    for hh in range(2):
                        ex = expp.tile([P, 2, T], f8, tag="exp",
                                       name=f"ex{hh}")
                        nc.scalar.activation(ex[:], scs[hh][:], AF.Exp)
                        exs.append(ex)
                    # fp8 DoubleRow context: one matmul contracts both
                    # key-token chunks of this group
                    for hh in range(2):
                        h = 2 * hp + hh
                        nc.tensor.matmul(
                            ps_ctx[hh][:], v_sb[:, so2:so2 + 2, h, 0:HD + 1],
                            exs[hh][:], perf_mode=DR,
                            start=(so2 == 0), stop=(so2 == SC - 2))

                def attn_evict(hp, hh, ps_ctx, ctxT, den_sb):
                    h = 2 * hp + hh
                    base = hh * HD
                    nc.vector.tensor_copy(
                        ctxT[base:base + HD, hp, :], ps_ctx[hh][0:HD, :])
                    nc.vector.tensor_copy(
                        den_sb[0:1, h, :], ps_ctx[hh][HD:HD + 1, :])

                with (
                    tc.tile_pool(name="psK", bufs=2, space="PSUM") as psK,
                    tc.tile_pool(name="psSC", bufs=2, space="PSUM") as psSC,
                    tc.tile_pool(name="psCTX", bufs=2, space="PSUM") as psCTX,
                ):
                    for hp in range(DC):
                        # K-projection for this head pair (no bias: softmax
                        # is invariant to the K bias)
                        ktch = attn_kproj(hp, kin, psK)
                        ps_ctx = [psCTX.tile([HD + 1, T], f32, tag="psCTX",
                                             name=f"ps_ctx{i}")
                                  for i in range(2)]
                        for so2 in range(0, SC, 2):
                            attn_scores_ctx(hp, so2, ktch, v_sb,
                                            ps_ctx, psSC)
                        for hh in range(2):
                            attn_evict(hp, hh, ps_ctx, ctxT, den_sb)

                # batch-wide softmax normalization: one reciprocal
                # instruction after all EXPs (single table load), per-head
                # partition broadcast via K=1 matmuls (PE is idle here)
                act_reciprocal(den_sb[0:1, :, :], den_sb[0:1, :, :])
                with tc.tile_pool(name="psBC", bufs=2, space="PSUM") as psBC:
                    for hp in range(DC):
                        ps_bc = psBC.tile([P, T], f32, tag="psBC")
                        for hh in range(2):
                            h = 2 * hp + hh
                            base = hh * HD
                            nc.tensor.matmul(
                                ps_bc[base:base + HD, :], ones64_bf[:],
                                den_sb[0:1, h, :],
                                start=True, stop=True,
                                tile_position=(0, base))
                        for hh in range(2):
                            base = hh * HD
                            nc.vector.tensor_tensor(
                                ctxT[base:base + HD, hp, :],
                                ctxT[base:base + HD, hp, :],
                                ps_bc[base:base + HD, :], ALU.mult)

                # ===== phase C: out_proj + residual + LN2 stats ==========
                xT = act.tile([P, DC, T], f32r, tag="act")
                hT = act.tile([P, DC, T], bf16, tag="act_h", bufs=1)
                with (
                    tc.tile_pool(name="psC", bufs=2, space="PSUM") as psC,
                    tc.tile_pool(name="psST", bufs=1, space="PSUM") as psST,
                ):
                    mu1 = psST.tile([1, T], f32, tag="st_mu")
                    sq1 = psST.tile([1, T], f32, tag="st_sq")
                    for mo in range(DC):
                        ps = psC.tile([P, T], f32, tag="psC")
                        for ki in range(DC):
                            nc.tensor.matmul(ps[:],
                                             wo_sb[:, ki, mo * P:(mo + 1) * P],
                                             ctxT[:, ki, :],
                                             start=(ki == 0), stop=(ki == DC - 1))
                        nc.vector.scalar_tensor_tensor(
                            xT[:, mo, :], ps[:], bo_pc[:, mo:mo + 1],
                            F(qin[:, mo, :]), op0=ALU.add, op1=ALU.add)
                        ln_stats_chunk(mo, xT, mu1, sq1)
                    ln_finish(xT, hT, ln2g_pc, ln2b_pc, ln2_triv, mu1, sq1)

                # ===== phase E: MLP (fused fc1->gelu->fc2) + LNp =========
                x2T = act.tile([P, DC, T], f32r, tag="act")
                outT = act.tile([P, DC, T], f32, tag="act")
                with tc.tile_pool(name="psF2", bufs=6, space="PSUM") as psF2:
                    ps_f2 = [psF2.tile([P, T], f32, tag="psF2", name=f"ps_f2_{i}")
                             for i in range(DC)]
                    with tc.tile_pool(name="psF1", bufs=2, space="PSUM") as psF1:
                        for fo in range(FFC):
                            f1_sl = fstream.tile([P, DC, P], bf16, tag="f1_sl")
                            nc.sync.dma_start(f1_sl[:], fc1_d.ap().rearrange(
                                "(k p) f -> p k f", p=P)[:, :, fo * P:(fo + 1) * P])
                            f2_sl = fstream.tile([P, D], bf16, tag="f2_sl")
                            nc.sync.dma_start(f2_sl[:], fc2_d.ap().rearrange(
                                "(ko p) o -> p ko o", p=P)[:, fo, :])
                            ps1 = psF1.tile([P, T], f32, tag="psF1")
                            for ki in range(DC):
                                nc.tensor.matmul(ps1[:], f1_sl[:, ki, :],
                                                 hT[:, ki, :],
                                                 start=(ki == 0),
                                                 stop=(ki == DC - 1))
                            mch = mchunkp.tile([P, T], bf16, tag="mch")
                            nc.scalar.activation(mch[:], ps1[:], AF.Gelu,
                                                 bias=f1b_pc[:, fo:fo + 1])
                            for mo in range(DC):
                                nc.tensor.matmul(
                                    ps_f2[mo][:], f2_sl[:, mo * P:(mo + 1) * P],
                                    mch[:],
                                    start=(fo == 0), stop=(fo == FFC - 1))
                    with tc.tile_pool(name="psST2", bufs=1,
                                      space="PSUM") as psST2:
                        mu2 = psST2.tile([1, T], f32, tag="st_mu")
                        sq2 = psST2.tile([1, T], f32, tag="st_sq")
                        for mo in range(DC):
                            nc.vector.scalar_tensor_tensor(
                                x2T[:, mo, :], ps_f2[mo][:],
                                f2b_pc[:, mo:mo + 1],
                                F(xT[:, mo, :]), op0=ALU.add, op1=ALU.add)
                            ln_stats_chunk(mo, x2T, mu2, sq2)
                        ln_finish(x2T, outT, lnpg_pc, lnpb_pc, lnp_triv,
                                  mu2, sq2)
                nc.sync.dma_start(
                    out_d.ap()[b].rearrange("(c p) t -> p c t", p=P), outT[:])

    nc.compile()
    return nc


def _get_nc(use_bv: bool, ln2_triv: bool, lnp_triv: bool):
    key = ("nc", use_bv, ln2_triv, lnp_triv)
    if key not in _cached:
        _cached[key] = _build(use_bv, ln2_triv, lnp_triv)
    return _cached[key]


def kernel(query, key, wq, bq, wk, bk, wv, bv, wo, bo,
           ln2_g, ln2_b, fc1_w, fc1_b, fc2_w, fc2_b, lnp_g, lnp_b):
    from concourse.bass_utils import run_bass_kernel_spmd
    import ml_dtypes

    f = np.float32
    bf = ml_dtypes.bfloat16
    c = np.ascontiguousarray

    def cbf(a):
        return np.ascontiguousarray(np.asarray(a, f).astype(bf))

    query = np.asarray(query, f)
    key = np.asarray(key, f)
    use_bv = bool(np.any(np.asarray(bv)))
    ln2_triv = (not np.any(np.asarray(ln2_b))) and bool(
        np.all(np.asarray(ln2_g) == 1.0))
    lnp_triv = (not np.any(np.asarray(lnp_b))) and bool(
        np.all(np.asarray(lnp_g) == 1.0))
    nc = _get_nc(use_bv, ln2_triv, lnp_triv)

    shared = {
        "wq_t": cbf(np.asarray(wq, f).T * np.float32(SCALE)),
        "wk_t": cbf(np.asarray(wk, f).T),
        "wv_t": cbf(np.asarray(wv, f).T),
        "wo_t": cbf(np.asarray(wo, f).T),
        "fc1_t": cbf(np.asarray(fc1_w, f).T),
        "fc2_t": cbf(np.asarray(fc2_w, f).T),
        "bqv": c(np.asarray(bq, f) * np.float32(SCALE)),
        "bvv": c(np.asarray(bv, f).reshape(1, D)),
        "bov": c(np.asarray(bo, f)),
        "f1b": c(np.asarray(fc1_b, f)),
        "f2b": c(np.asarray(fc2_b, f)),
        "ln2g": c(np.asarray(ln2_g, f)),
        "ln2b": c(np.asarray(ln2_b, f)),
        "lnpg": c(np.asarray(lnp_g, f)),
        "lnpb": c(np.asarray(lnp_b, f)),
        "ones_col": np.ones((P, 1), f),
    }
    in_maps = []
    for core in range(NCORES):
        sl = slice(core * BPC, (core + 1) * BPC)
        m = dict(shared)
        qt_t = query[sl].transpose(0, 2, 1)
        m["qT"] = c(qt_t)
        m["qTb"] = np.ascontiguousarray(qt_t.astype(bf))
        m["kT"] = np.ascontiguousarray(
            key[sl].transpose(0, 2, 1).astype(bf))
        in_maps.append(m)

    res = run_bass_kernel_spmd(nc, in_maps, core_ids=list(range(NCORES)))
    kernel._last_result = res
    out = np.concatenate([r["out"] for r in res.results], axis=0)
    return c(out.transpose(0, 2, 1))
